# revision 3
# baseline (speedup 1.0000x reference)
"""Trainium kernel for nn_BackBone_20074677141869 (gnn_message_passing).

Strategy (8 NeuronCores, full I/O):
  - Stage 1 (pmap, T-sharded 3 t/core): per-timestep bipartite GINE conv
    (2 layers) -> enc_a [24,128,128], enc_d [24,6000,128]
  - Stage 2 (pmap, batch-sharded): causal transformer decoder; drugs 750/core,
    admissions 16/core.
  - Stage 3 (pmap, T-sharded): per-day link prediction on sampled pairs.
  Host only reshuffles shards between stages (no float math on host).
"""
import numpy as np
import jax
import jax.numpy as jnp
from functools import partial

T, NA, ND, E, L = 24, 128, 6000, 20000, 8000
DA, DD, DE, H, NH, FF = 64, 64, 16, 128, 8, 512
NL_GNN, NL_DEC = 2, 2
NC = 8  # cores
TS = T // NC   # 3 timesteps per core
DS = ND // NC  # 750 drugs per core
AS = NA // NC  # 16 admissions per core


def _posenc(t, h):
    pos = jnp.arange(t, dtype=jnp.float32)[:, None]
    div = jnp.exp(-np.log(10000.0) * jnp.arange(0, h, 2, dtype=jnp.float32) / h)
    pe = jnp.zeros((t, h), jnp.float32)
    pe = pe.at[:, 0::2].set(jnp.sin(pos * div))
    pe = pe.at[:, 1::2].set(jnp.cos(pos * div))
    return pe


def _ln(x, g, b):
    m = x.mean(-1, keepdims=True)
    v = ((x - m) ** 2).mean(-1, keepdims=True)
    return (x - m) * jax.lax.rsqrt(v + 1e-5) * g + b


def _causal_mha(x, wq, wk, wv, wo):
    t, n, h = x.shape
    hd = h // NH
    q = (x @ wq).reshape(t, n, NH, hd)
    k = (x @ wk).reshape(t, n, NH, hd)
    v = (x @ wv).reshape(t, n, NH, hd)
    scores = jnp.einsum('tnhd,snhd->nhts', q, k) / np.float32(np.sqrt(hd))
    mask = jnp.tril(jnp.ones((t, t), bool))
    scores = jnp.where(mask, scores, jnp.float32(-1e9))
    a = jax.nn.softmax(scores, axis=-1)
    o = jnp.einsum('nhts,snhd->tnhd', a, v).reshape(t, n, h)
    return o @ wo


def _decode(x, h0, dp):
    for l in range(NL_DEC):
        x = _ln(x + _causal_mha(x, dp['Wq'][l], dp['Wk'][l], dp['Wv'][l], dp['Wo'][l]),
                dp['g1'][l], dp['b1'][l])
        cross = (h0 @ dp['Wvc'][l]) @ dp['Woc'][l]
        x = _ln(x + cross[None], dp['g2'][l], dp['b2'][l])
        x = _ln(x + jax.nn.relu(x @ dp['W1'][l]) @ dp['W2'][l], dp['g3'][l], dp['b3'][l])
    return x


# ---------------- stage 1: base features + GNN over my 3 timesteps ----------
# Edges arrive pre-sorted by dst-tile (128 drugs per tile, KT=47 tiles) with
# -1 padding. Scatter-adds are one-hot matmuls (Neuron runtime crashes on
# large native scatters); gathers are native (they work).
KT = (ND + 127) // 128  # 47 dst tiles


def _stage1(ea_s, src_s, dstloc_s, x_adm, x_drug, emb_g, p):
    # ea_s [TS,KT,W,16]; src_s/dstloc_s [TS,KT,W] int32 (-1 = pad)
    ha0 = x_adm @ p['W_adm'] + p['b_adm']
    hd0 = x_drug @ p['W_drug'] + p['b_drug'] + emb_g
    g = p['gnn']
    iota128 = jnp.arange(128, dtype=jnp.int32)

    def step(e_t, src, dstloc):
        e = e_t @ p['W_e'] + p['b_e']                     # [KT,W,H]
        srcc = jnp.clip(src, 0, NA - 1)
        dstc = jnp.clip(dstloc, 0, 127)
        oh_d = (dstloc[:, None, :] == iota128[None, :, None]).astype(jnp.float32)  # [KT,128,W]
        oh_a = (src.reshape(-1)[None, :] == iota128[:, None]).astype(jnp.float32)  # [128,KT*W]
        ha, hd = ha0, hd0
        for l in range(NL_GNN):
            hd_pad = jnp.pad(hd, ((0, KT * 128 - ND), (0, 0))).reshape(KT, 128, H)
            z_d = jax.nn.relu(jnp.take(ha, srcc, axis=0) + e)               # [KT,W,H]
            z_a = jax.nn.relu(jnp.take_along_axis(hd_pad, dstc[..., None], axis=1) + e)
            m_d = jnp.einsum('kpw,kwh->kph', oh_d, z_d).reshape(KT * 128, H)[:ND]
            m_a = oh_a @ z_a.reshape(-1, H)                                 # [128,H]
            ha = jax.nn.relu(ha @ g['Wsa'][l] + m_a @ g['Wma'][l])
            hd = jax.nn.relu(hd @ g['Wsd'][l] + m_d @ g['Wmd'][l])
        return ha, hd

    enc_a, enc_d = jax.vmap(step)(ea_s, src_s, dstloc_s)  # [TS,NA,H],[TS,ND,H]
    return enc_a, enc_d, ha0, hd0


# ---------------- stage 2: decoders on batch shards -------------------------
def _stage2(enc_a_sl, enc_d_sl, ha0_sl, hd0_sl, p):
    pe = _posenc(T, H)
    dec_a = _decode(enc_a_sl + pe[:, None, :], ha0_sl, p['dec_adm'])
    dec_d = _decode(enc_d_sl + pe[:, None, :], hd0_sl, p['dec_drug'])
    return dec_a, dec_d


# ---------------- stage 3: link prediction on my 3 timesteps ----------------
def _stage3(dec_a_sl, dec_d_sl, ls, ld, w_lp):
    u = jnp.take_along_axis(dec_a_sl, ls[..., None], axis=1)
    v = jnp.take_along_axis(dec_d_sl, ld[..., None], axis=1)
    return jnp.einsum('tlh,h->tl', u * v, w_lp)


_s1 = None
_s2 = None
_s3 = None


def _build():
    global _s1, _s2, _s3
    if _s1 is None:
        _s1 = jax.pmap(_stage1, in_axes=(0, 0, 0, None, None, None, None))
        _s2 = jax.pmap(_stage2, in_axes=(0, 0, 0, 0, None))
        _s3 = jax.pmap(_stage3, in_axes=(0, 0, 0, 0, None))
    return _s1, _s2, _s3


def kernel(x_adm, x_drug, edge_attr, params, drug_ids, edge_src, edge_dst,
           labels_src, labels_dst):
    s1, s2, s3 = _build()
    x_adm = np.asarray(x_adm, np.float32)
    x_drug = np.asarray(x_drug, np.float32)
    edge_attr = np.asarray(edge_attr, np.float32)
    drug_ids = np.asarray(drug_ids, np.int32)
    edge_src = np.asarray(edge_src, np.int32)
    edge_dst = np.asarray(edge_dst, np.int32)
    labels_src = np.asarray(labels_src, np.int32)
    labels_dst = np.asarray(labels_dst, np.int32)
    p = jax.tree.map(lambda a: np.asarray(a, np.float32), params)
    emb_g = p['emb'][drug_ids]  # host gather of embedding rows (arange ids)

    # ---- stage 1: shard over T -------------------------------------------
    # host prep: group each timestep's edges by dst tile (index work only)
    tile_of = edge_dst // 128                     # [T,E]
    counts = np.zeros((T, KT), np.int64)
    for t in range(T):
        np.add.at(counts[t], tile_of[t], 1)
    W = int(-(-counts.max() // 64) * 64)          # pad group width to mult of 64
    ea_s = np.zeros((T, KT, W, DE), np.float32)
    src_s = np.full((T, KT, W), -1, np.int32)
    dst_s = np.full((T, KT, W), -1, np.int32)
    for t in range(T):
        order = np.argsort(tile_of[t], kind='stable')
        pos = np.concatenate([[0], np.cumsum(counts[t])[:-1]])
        idx_in_grp = np.arange(E) - pos[tile_of[t][order]]
        k = tile_of[t][order]
        ea_s[t, k, idx_in_grp] = edge_attr[t, order]
        src_s[t, k, idx_in_grp] = edge_src[t, order]
        dst_s[t, k, idx_in_grp] = edge_dst[t, order] - k * 128
    enc_a_sh, enc_d_sh, ha0_r, hd0_r = s1(
        ea_s.reshape(NC, TS, KT, W, DE), src_s.reshape(NC, TS, KT, W),
        dst_s.reshape(NC, TS, KT, W), x_adm, x_drug, emb_g, p)
    enc_a = np.asarray(enc_a_sh).reshape(T, NA, H)
    enc_d = np.asarray(enc_d_sh).reshape(T, ND, H)
    ha0 = np.asarray(ha0_r[0])
    hd0 = np.asarray(hd0_r[0])

    # ---- stage 2: shard decoder batch ------------------------------------
    enc_a_sl = np.stack([enc_a[:, c * AS:(c + 1) * AS] for c in range(NC)])
    enc_d_sl = np.stack([enc_d[:, c * DS:(c + 1) * DS] for c in range(NC)])
    ha0_sl = np.stack([ha0[c * AS:(c + 1) * AS] for c in range(NC)])
    hd0_sl = np.stack([hd0[c * DS:(c + 1) * DS] for c in range(NC)])
    dec_a_sh, dec_d_sh = s2(enc_a_sl, enc_d_sl, ha0_sl, hd0_sl, p)
    dec_a = np.concatenate(list(np.asarray(dec_a_sh)), axis=1)  # [T,NA,H]
    dec_d = np.concatenate(list(np.asarray(dec_d_sh)), axis=1)  # [T,ND,H]

    # ---- stage 3: shard over T -------------------------------------------
    da = dec_a.reshape(NC, TS, NA, H)
    dd = dec_d.reshape(NC, TS, ND, H)
    ls = labels_src.reshape(NC, TS, L)
    ld = labels_dst.reshape(NC, TS, L)
    scores_sh = s3(da, dd, ls, ld, p['w_lp'])
    return np.asarray(scores_sh).reshape(T, L)


# revision 9
# speedup vs baseline: 1.5086x; 1.5086x over previous
"""Trainium kernel for nn_BackBone_20074677141869 (gnn_message_passing).

Strategy (8 NeuronCores, full I/O):
  - Stage 1 (pmap, T-sharded 3 t/core): per-timestep bipartite GINE conv
    (2 layers) -> enc_a [24,128,128], enc_d [24,6000,128]
  - Stage 2 (pmap, batch-sharded): causal transformer decoder; drugs 750/core,
    admissions 16/core.
  - Stage 3 (pmap, T-sharded): per-day link prediction on sampled pairs.
  Host only reshuffles shards between stages (no float math on host).
"""
import numpy as np
import jax
import jax.numpy as jnp
from functools import partial

T, NA, ND, E, L = 24, 128, 6000, 20000, 8000
DA, DD, DE, H, NH, FF = 64, 64, 16, 128, 8, 512
NL_GNN, NL_DEC = 2, 2
NC = 8  # cores
TS = T // NC   # 3 timesteps per core
DS = ND // NC  # 750 drugs per core
AS = NA // NC  # 16 admissions per core


def _posenc(t, h):
    pos = jnp.arange(t, dtype=jnp.float32)[:, None]
    div = jnp.exp(-np.log(10000.0) * jnp.arange(0, h, 2, dtype=jnp.float32) / h)
    pe = jnp.zeros((t, h), jnp.float32)
    pe = pe.at[:, 0::2].set(jnp.sin(pos * div))
    pe = pe.at[:, 1::2].set(jnp.cos(pos * div))
    return pe


def _ln(x, g, b):
    m = x.mean(-1, keepdims=True)
    v = ((x - m) ** 2).mean(-1, keepdims=True)
    return (x - m) * jax.lax.rsqrt(v + 1e-5) * g + b


def _causal_mha(x, wq, wk, wv, wo):
    t, n, h = x.shape
    hd = h // NH
    q = (x @ wq).reshape(t, n, NH, hd)
    k = (x @ wk).reshape(t, n, NH, hd)
    v = (x @ wv).reshape(t, n, NH, hd)
    scores = jnp.einsum('tnhd,snhd->nhts', q, k) / np.float32(np.sqrt(hd))
    mask = jnp.tril(jnp.ones((t, t), bool))
    scores = jnp.where(mask, scores, jnp.float32(-1e9))
    a = jax.nn.softmax(scores, axis=-1)
    o = jnp.einsum('nhts,snhd->tnhd', a, v).reshape(t, n, h)
    return o @ wo


def _decode(x, h0, dp):
    for l in range(NL_DEC):
        x = _ln(x + _causal_mha(x, dp['Wq'][l], dp['Wk'][l], dp['Wv'][l], dp['Wo'][l]),
                dp['g1'][l], dp['b1'][l])
        cross = (h0 @ dp['Wvc'][l]) @ dp['Woc'][l]
        x = _ln(x + cross[None], dp['g2'][l], dp['b2'][l])
        x = _ln(x + jax.nn.relu(x @ dp['W1'][l]) @ dp['W2'][l], dp['g3'][l], dp['b3'][l])
    return x


# ---------------- stage 1: base features + GNN over my 3 timesteps ----------
# Edges arrive pre-sorted by dst-tile (128 drugs per tile, KT=47 tiles) with
# -1 padding. Scatter-adds are one-hot matmuls (Neuron runtime crashes on
# large native scatters); gathers are native (they work).
KT = (ND + 127) // 128  # 47 dst tiles


def _stage1(ea_s, src_s, dstloc_s, x_adm, x_drug, emb_g, p):
    # ea_s [TS,KT,W,16]; src_s/dstloc_s [TS,KT,W] int32 (-1 = pad)
    ha0 = x_adm @ p['W_adm'] + p['b_adm']
    hd0 = x_drug @ p['W_drug'] + p['b_drug'] + emb_g
    g = p['gnn']
    iota128 = jnp.arange(128, dtype=jnp.int32)

    def step(e_t, src, dstloc):
        e = e_t @ p['W_e'] + p['b_e']                     # [KT,W,H]
        srcc = jnp.clip(src, 0, NA - 1)
        dstc = jnp.clip(dstloc, 0, 127)
        oh_d = (dstloc[:, None, :] == iota128[None, :, None]).astype(jnp.float32)  # [KT,128,W]
        oh_a = (src.reshape(-1)[None, :] == iota128[:, None]).astype(jnp.float32)  # [128,KT*W]
        ha, hd = ha0, hd0
        for l in range(NL_GNN):
            hd_pad = jnp.pad(hd, ((0, KT * 128 - ND), (0, 0))).reshape(KT, 128, H)
            z_d = jax.nn.relu(jnp.take(ha, srcc, axis=0) + e)               # [KT,W,H]
            z_a = jax.nn.relu(jnp.take_along_axis(hd_pad, dstc[..., None], axis=1) + e)
            m_d = jnp.einsum('kpw,kwh->kph', oh_d, z_d).reshape(KT * 128, H)[:ND]
            m_a = oh_a @ z_a.reshape(-1, H)                                 # [128,H]
            ha = jax.nn.relu(ha @ g['Wsa'][l] + m_a @ g['Wma'][l])
            hd = jax.nn.relu(hd @ g['Wsd'][l] + m_d @ g['Wmd'][l])
        return ha, hd

    enc_a, enc_d = jax.vmap(step)(ea_s, src_s, dstloc_s)  # [TS,NA,H],[TS,ND,H]
    return enc_a, enc_d, ha0, hd0


# ---------------- stage 2 (fused): decoders + partial link prediction -------
# Admission decoder replicated (tiny); drug decoder on my 750-drug shard;
# each core scores only labels whose drug lives in its shard (host sums).
def _stage2(enc_a, enc_d_sl, ha0, hd0_sl, ls, ld, base, p):
    pe = _posenc(T, H)
    dec_a = _decode(enc_a + pe[:, None, :], ha0, p['dec_adm'])          # [T,NA,H]
    dec_d = _decode(enc_d_sl + pe[:, None, :], hd0_sl, p['dec_drug'])   # [T,DS,H]
    u = jnp.take_along_axis(dec_a, ls[..., None], axis=1)               # [T,L,H]
    ld_loc = ld - base
    valid = (ld_loc >= 0) & (ld_loc < DS)
    v = jnp.take_along_axis(dec_d, jnp.clip(ld_loc, 0, DS - 1)[..., None], axis=1)
    scores = jnp.einsum('tlh,h->tl', u * v, p['w_lp'])
    return jnp.where(valid, scores, 0.0)                                # [T,L]


_s1 = None
_s2 = None


def _build():
    global _s1, _s2, _s3
    if _s1 is None:
        _s1 = jax.pmap(_stage1, in_axes=(0, 0, 0, None, None, None, None))
        _s2 = jax.pmap(_stage2, in_axes=(None, 0, None, 0, None, None, 0, None))
    return _s1, _s2


def kernel(x_adm, x_drug, edge_attr, params, drug_ids, edge_src, edge_dst,
           labels_src, labels_dst):
    s1, s2 = _build()
    x_adm = np.asarray(x_adm, np.float32)
    x_drug = np.asarray(x_drug, np.float32)
    edge_attr = np.asarray(edge_attr, np.float32)
    drug_ids = np.asarray(drug_ids, np.int32)
    edge_src = np.asarray(edge_src, np.int32)
    edge_dst = np.asarray(edge_dst, np.int32)
    labels_src = np.asarray(labels_src, np.int32)
    labels_dst = np.asarray(labels_dst, np.int32)
    p = jax.tree.map(lambda a: np.asarray(a, np.float32), params)
    emb_g = p['emb'][drug_ids]  # host gather of embedding rows (arange ids)

    # ---- stage 1: shard over T -------------------------------------------
    # host prep: group each timestep's edges by dst tile (index work only)
    tile_of = edge_dst // 128                     # [T,E]
    counts = np.zeros((T, KT), np.int64)
    for t in range(T):
        np.add.at(counts[t], tile_of[t], 1)
    W = int(-(-counts.max() // 64) * 64)          # pad group width to mult of 64
    ea_s = np.zeros((T, KT, W, DE), np.float32)
    src_s = np.full((T, KT, W), -1, np.int32)
    dst_s = np.full((T, KT, W), -1, np.int32)
    for t in range(T):
        order = np.argsort(tile_of[t], kind='stable')
        pos = np.concatenate([[0], np.cumsum(counts[t])[:-1]])
        idx_in_grp = np.arange(E) - pos[tile_of[t][order]]
        k = tile_of[t][order]
        ea_s[t, k, idx_in_grp] = edge_attr[t, order]
        src_s[t, k, idx_in_grp] = edge_src[t, order]
        dst_s[t, k, idx_in_grp] = edge_dst[t, order] - k * 128
    enc_a_sh, enc_d_sh, ha0_r, hd0_r = s1(
        ea_s.reshape(NC, TS, KT, W, DE), src_s.reshape(NC, TS, KT, W),
        dst_s.reshape(NC, TS, KT, W), x_adm, x_drug, emb_g, p)
    enc_a = np.asarray(enc_a_sh).reshape(T, NA, H)
    enc_d = np.asarray(enc_d_sh).reshape(T, ND, H)
    ha0 = np.asarray(ha0_r[0])
    hd0 = np.asarray(hd0_r[0])

    # ---- stage 2 (fused decoders + partial link-pred): shard drug batch --
    enc_d_sl = np.stack([enc_d[:, c * DS:(c + 1) * DS] for c in range(NC)])
    hd0_sl = np.stack([hd0[c * DS:(c + 1) * DS] for c in range(NC)])
    base = np.arange(NC, dtype=np.int32) * DS
    scores_part = s2(enc_a, enc_d_sl, ha0, hd0_sl, labels_src, labels_dst, base, p)
    return np.asarray(scores_part).sum(axis=0).astype(np.float32)  # [T,L]
